# revision 11
# baseline (speedup 1.0000x reference)
"""AspectMemorySummarizer on 8 TRN2 NeuronCores.

Math (per example b): gather word embeddings x = w_emb[ids[b]], cosine-sim
against 30 aspect vectors, threshold > 0.2, scale by aspect weight, max over
aspects -> per-token score M.  centroid_score = sum(M)/len; attention =
softmax over tokens of (M if M>0 else -1e9); z = attn @ x;
enc = z * (centroid_score > 1e-4).

Key restructuring: M depends only on the vocab id, so it is precomputed per
vocab word (V-sharded across the 8 cores + AllGather), then fetched per token.
The embedding gather uses a per-core deduplicated table (unique ids of the
core's batch shard, <= 32768 rows) so indices fit dma_gather's int16 and rows
can be padded to a 256B multiple in bf16.

Sharding: data-parallel over B for the main phase (128 examples/core);
V-sharded phase 1 (6250 vocab rows/core) + 200KB AllGather.
"""
import sys, os
sys.path.insert(0, "/opt/trn_rl_repo")
import numpy as np
import concourse.bass as bass
import concourse.bacc as bacc
import concourse.mybir as mybir
from concourse.bass import IndirectOffsetOnAxis
from concourse.tile import TileContext
from concourse.bass_utils import run_bass_kernel_spmd

F32 = mybir.dt.float32
BF16 = mybir.dt.bfloat16
I16 = mybir.dt.int16
I32 = mybir.dt.int32
NP_BF16 = mybir.dt.np(BF16)


def _install_ntff_hook_shim():
    """The agent image's antenv lacks axon_hooks; provide it so
    run_bass_kernel_spmd(trace=True) can capture NTFF profiles."""
    import types
    try:
        import antenv
        try:
            from antenv import axon_hooks  # noqa: F401
            return  # already present
        except ImportError:
            pass
        mod = types.ModuleType("antenv.axon_hooks")
        mod._hook = None

        def set_axon_ntff_profile_hook(h):
            mod._hook = h

        def get_axon_ntff_profile_hook():
            return mod._hook

        mod.set_axon_ntff_profile_hook = set_axon_ntff_profile_hook
        mod.get_axon_ntff_profile_hook = get_axon_ntff_profile_hook
        sys.modules["antenv.axon_hooks"] = mod
        antenv.axon_hooks = mod
        from trn_agent_boot.trn_boot import _ntff_profile_via_ctypes
        hook = _ntff_profile_via_ctypes("/opt/axon/libaxon_pjrt.so")
        if hook is not None:
            mod.set_axon_ntff_profile_hook(hook)
    except Exception:
        pass


_install_ntff_hook_shim()


B, L, V, E, A = 1024, 256, 50000, 300, 30
N_CORES = 8
BS = B // N_CORES          # 128 examples per core
VS = V // N_CORES          # 6250 vocab rows per core (phase 1)
VT = 49                    # phase-1 v-tiles of 128 (6272 padded)
VS_PAD = VT * 128
NU = 32768                 # unique-table rows (>= max distinct ids per core)
EP = 384                   # bf16 row padding: 384*2 = 768 B (256B multiple)
NT = BS * L                # 32768 token instances per core
LC = 32                    # l-chunk size for the gather/attention pipeline
NCHUNK = L // LC
XG_BUFS = 4                # gathered-x chunk buffers resident in SBUF

WORD_THRES = 0.2
COS_EPS = 1e-8
MASK_NEG = -80.0           # exp(-80) ~ 1.8e-35: matches softmax(-1e9) to f32
STAGE = int(os.environ.get("K_STAGE", "3"))  # 1: no gathers/z  2: +x-gathers  3: full
SUB = os.environ.get("K_SUB", "")  # "noag": skip AllGather+Mpath; "nom": skip M-gathers only


def _wrap16(flat_idx: np.ndarray) -> np.ndarray:
    """dma_gather index layout: index i lives at [i % 16, i // 16],
    replicated across the 8 gpsimd cores (-> 128 partitions)."""
    n = flat_idx.shape[0]
    arr = flat_idx.reshape(n // 16, 16).T
    return np.tile(arr, (8, 1)).copy()


def _build():
    nc = bacc.Bacc()

    # ---- inputs (per core) ----
    w_vmaj = nc.declare_dram_parameter("w_vmaj", [VS_PAD, E], F32, isOutput=False)
    w_emaj = nc.declare_dram_parameter("w_emaj", [E, VS_PAD], F32, isOutput=False)
    a_mat = nc.declare_dram_parameter("a_mat", [A, E], F32, isOutput=False)
    a_w = nc.declare_dram_parameter("a_w", [1, A], F32, isOutput=False)
    ut = nc.declare_dram_parameter("ut", [NU, EP], BF16, isOutput=False)
    idx16 = nc.declare_dram_parameter("idx16", [128, NT // 16], I16, isOutput=False)
    idxm16 = nc.declare_dram_parameter("idxm16", [128, NT // 16], I16, isOutput=False)
    oh = nc.declare_dram_parameter("oh", [BS, L * 64], F32, isOutput=False)
    lens = nc.declare_dram_parameter("lens", [BS, 1], F32, isOutput=False)

    enc_o = nc.declare_dram_parameter("enc", [BS, E], F32, isOutput=True)
    attn_o = nc.declare_dram_parameter("attn", [BS, L], F32, isOutput=True)
    cs_o = nc.declare_dram_parameter("cscore", [BS, 1], F32, isOutput=True)

    m_shard = nc.dram_tensor("m_shard", [VS_PAD], F32)
    m_full = nc.dram_tensor("m_full", [784 * 64], F32)

    ident_f = nc.inline_tensor(np.eye(128, dtype=np.float32), name="ident_f")
    ident_b = nc.inline_tensor(np.eye(128, dtype=NP_BF16), name="ident_b")

    with TileContext(nc) as tc:
        with (
            tc.tile_pool(name="const", bufs=1) as cpool,
            tc.tile_pool(name="p1", bufs=3) as p1,
            tc.tile_pool(name="p1s", bufs=4) as p1s,
            tc.tile_pool(name="xg", bufs=XG_BUFS) as xgp,
            tc.tile_pool(name="p2", bufs=1) as p2,
            tc.tile_pool(name="mb", bufs=2) as mbp,
            tc.tile_pool(name="diag", bufs=4) as dgp,
            tc.tile_pool(name="psum", bufs=2, space="PSUM") as pp,
            tc.tile_pool(name="psz", bufs=1, space="PSUM") as ppz,
        ):
            # ---------- constants ----------
            idf = cpool.tile([128, 128], F32)
            nc.sync.dma_start(out=idf[:], in_=ident_f[:, :])
            idb = cpool.tile([128, 128], BF16)
            nc.sync.dma_start(out=idb[:], in_=ident_b[:, :])
            idx_sb = cpool.tile([128, NT // 16], I16)
            nc.sync.dma_start(out=idx_sb[:], in_=idx16[:, :])
            idxm_sb = cpool.tile([128, NT // 16], I16)
            nc.sync.dma_start(out=idxm_sb[:], in_=idxm16[:, :])
            lens_sb = cpool.tile([BS, 1], F32)
            nc.sync.dma_start(out=lens_sb[:], in_=lens[:, :])

            # ---------- x-gathers (independent of phase 1; issue early) ----------
            # Only XG_BUFS gathers are issued before the collective: a later
            # gather waits on a buffer slot released by z-matmuls, which need
            # M_tok from the indirect gather queued on this same engine --
            # issuing all 8 up front deadlocks the Pool queue.
            xg_tiles = [None] * NCHUNK

            def issue_gather(c):
                xg = xgp.tile([128, LC, EP], BF16, tag="xg")
                nidx = 128 * LC
                nc.gpsimd.dma_gather(
                    xg[:], ut[:, :], idx_sb[:, c * (nidx // 16):(c + 1) * (nidx // 16)],
                    num_idxs=nidx, num_idxs_reg=nidx, elem_size=EP,
                    single_packet=False,
                )
                xg_tiles[c] = xg

            if STAGE >= 2:
                for c in range(min(XG_BUFS, NCHUNK)):
                    issue_gather(c)

            # ---------- aspect preprocessing ----------
            am = cpool.tile([A, E], F32)
            nc.sync.dma_start(out=am[:], in_=a_mat[:, :])
            n2a = cpool.tile([A, 1], F32)
            sc = cpool.tile([A, E], F32, tag="ascaled")
            nc.vector.tensor_tensor(out=sc[:], in0=am[:], in1=am[:],
                                    op=mybir.AluOpType.mult)
            nc.vector.reduce_sum(n2a[:], sc[:], axis=mybir.AxisListType.X)
            na = cpool.tile([A, 1], F32, tag="na")
            nc.scalar.activation(na[:], n2a[:], mybir.ActivationFunctionType.Sqrt)
            nc.vector.tensor_scalar_max(na[:], na[:], COS_EPS)
            ra = cpool.tile([A, 1], F32, tag="ra")
            nc.vector.reciprocal(ra[:], na[:])
            asc = cpool.tile([A, E], F32, tag="asc")
            nc.vector.tensor_scalar_mul(asc[:], am[:], ra[:])
            # transpose scaled aspects -> asT chunks [e_chunk, A]
            e_chunks = [(0, 128), (128, 128), (256, E - 256)]
            asT = []
            for k, (e0, ew) in enumerate(e_chunks):
                pt = pp.tile([128, A], F32, tag="asT_ps")
                nc.tensor.transpose(pt[:ew, :], asc[:, e0:e0 + ew], idf[:A, :A])
                st = cpool.tile([128, A], F32, tag=f"asT{k}")
                nc.vector.tensor_copy(st[:ew, :], pt[:ew, :])
                asT.append(st)
            # aspect weights replicated across partitions: ones[128,1] @ a_w[1,A]
            aw_sb = cpool.tile([1, A], F32)
            nc.sync.dma_start(out=aw_sb[:], in_=a_w[:, :])
            ones_row = cpool.tile([1, 128], F32, tag="ones_row")
            nc.vector.memset(ones_row[:], 1.0)
            wrep_ps = pp.tile([128, A], F32, tag="wrep_ps")
            nc.tensor.matmul(wrep_ps[:], lhsT=ones_row[:], rhs=aw_sb[:], start=True, stop=True)
            wrep = cpool.tile([128, A], F32, tag="wrep")
            nc.vector.tensor_copy(wrep[:], wrep_ps[:])

            # ---------- phase 1: M per vocab word (V-shard) ----------
            for t in range(VT):
                wv = p1.tile([128, E], F32, tag="wv")
                nc.sync.dma_start(out=wv[:], in_=w_vmaj[t * 128:(t + 1) * 128, :])
                wT = []
                for k, (e0, ew) in enumerate(e_chunks):
                    wt = p1.tile([128, 128], F32, tag=f"wT{k}")
                    nc.sync.dma_start(
                        out=wt[:ew, :], in_=w_emaj[e0:e0 + ew, t * 128:(t + 1) * 128])
                    wT.append(wt)
                n2 = p1s.tile([128, 1], F32, tag="n2")
                scr = p1s.tile([128, E], F32, tag="scr")
                nc.vector.tensor_tensor(out=scr[:], in0=wv[:], in1=wv[:],
                                        op=mybir.AluOpType.mult)
                nc.vector.reduce_sum(n2[:], scr[:], axis=mybir.AxisListType.X)
                xn = p1s.tile([128, 1], F32, tag="xn")
                nc.scalar.activation(xn[:], n2[:], mybir.ActivationFunctionType.Sqrt)
                nc.vector.tensor_scalar_max(xn[:], xn[:], COS_EPS)
                rx = p1s.tile([128, 1], F32, tag="rx")
                nc.vector.reciprocal(rx[:], xn[:])

                dot = pp.tile([128, A], F32, tag="dot")
                for k, (e0, ew) in enumerate(e_chunks):
                    nc.tensor.matmul(
                        dot[:], lhsT=wT[k][:ew, :], rhs=asT[k][:ew, :],
                        start=(k == 0), stop=(k == 2),
                    )
                cosv = p1s.tile([128, A], F32, tag="cosv")
                nc.vector.tensor_scalar_mul(cosv[:], dot[:], rx[:])
                mask = p1s.tile([128, A], F32, tag="mask")
                nc.vector.tensor_scalar(
                    out=mask[:], in0=cosv[:], scalar1=WORD_THRES, scalar2=None,
                    op0=mybir.AluOpType.is_gt,
                )
                tw = p1s.tile([128, A], F32, tag="tw")
                nc.vector.tensor_tensor(out=tw[:], in0=cosv[:], in1=wrep[:],
                                        op=mybir.AluOpType.mult)
                nc.vector.tensor_tensor(out=tw[:], in0=tw[:], in1=mask[:],
                                        op=mybir.AluOpType.mult)
                mt = p1s.tile([128, 1], F32, tag="mt")
                nc.vector.reduce_max(mt[:], tw[:], axis=mybir.AxisListType.X)
                nc.sync.dma_start(
                    out=m_shard[t * 128:(t + 1) * 128].rearrange("(p o) -> p o", p=128),
                    in_=mt[:],
                )

            if SUB != "noag":
                nc.gpsimd.collective_compute(
                    "AllGather", mybir.AluOpType.bypass,
                    replica_groups=[list(range(N_CORES))],
                    ins=[m_shard[0:VS].opt()],
                    outs=[m_full[0:V].opt()],
                )
                zpad = p2.tile([1, 784 * 64 - V], F32, tag="zpad")
                nc.vector.memset(zpad[:], 0.0)
                nc.sync.dma_start(
                    out=m_full[V:784 * 64].rearrange("(p f) -> p f", p=1), in_=zpad[:])

            # ---------- per-token M: gather 64-wide blocks, onehot select ----------
            # (indirect DMA only supports one index per partition row, so a
            # scalar gather is done as block-gather + in-block select)
            m_tok = p2.tile([BS, L], F32, tag="m_tok")
            if SUB in ("noag", "nom"):
                nc.vector.memset(m_tok[:], 0.0)
            MC = 64                      # l's per M-chunk
            m_in = m_full.ap().rearrange("(v c) -> v c", c=64)
            for mc in range(L // MC if SUB not in ("noag", "nom") else 0):
                nidx = 128 * MC
                mblk = mbp.tile([128, MC, 64], F32, tag="mblk")
                nc.gpsimd.dma_gather(
                    mblk[:], m_in,
                    idxm_sb[:, mc * (nidx // 16):(mc + 1) * (nidx // 16)],
                    num_idxs=nidx, num_idxs_reg=nidx, elem_size=64,
                    single_packet=False,
                )
                oh_sb = mbp.tile([BS, MC * 64], F32, tag="oh_sb")
                nc.sync.dma_start(
                    out=oh_sb[:], in_=oh[:, mc * MC * 64:(mc + 1) * MC * 64])
                nc.vector.tensor_tensor(
                    out=mblk[:], in0=mblk[:],
                    in1=oh_sb[:].rearrange("p (a b) -> p a b", b=64),
                    op=mybir.AluOpType.mult)
                nc.vector.reduce_max(
                    m_tok[:, mc * MC:(mc + 1) * MC], mblk[:],
                    axis=mybir.AxisListType.X)

            # ---------- softmax pieces ----------
            maskp = p2.tile([BS, L], F32, tag="maskp")
            nc.vector.tensor_scalar(
                out=maskp[:], in0=m_tok[:], scalar1=0.0, scalar2=None,
                op0=mybir.AluOpType.is_gt,
            )
            # score = M + (mask*80 - 80):  masked tokens -> M-80 ~= -80
            adj = p2.tile([BS, L], F32, tag="adj")
            nc.vector.tensor_scalar(
                out=adj[:], in0=maskp[:], scalar1=-MASK_NEG, scalar2=MASK_NEG,
                op0=mybir.AluOpType.mult, op1=mybir.AluOpType.add,
            )
            score = p2.tile([BS, L], F32, tag="score")
            nc.vector.tensor_tensor(out=score[:], in0=m_tok[:], in1=adj[:],
                                    op=mybir.AluOpType.add)
            expv = p2.tile([BS, L], F32, tag="expv")
            den = p2.tile([BS, 1], F32, tag="den")
            nc.scalar.activation(expv[:], score[:], mybir.ActivationFunctionType.Exp,
                                 accum_out=den[:])
            rden = p2.tile([BS, 1], F32, tag="rden")
            nc.vector.reciprocal(rden[:], den[:])
            # centroid score + gate
            csum = p2.tile([BS, 1], F32, tag="csum")
            nc.vector.reduce_sum(csum[:], m_tok[:], axis=mybir.AxisListType.X)
            lc_ = p2.tile([BS, 1], F32, tag="lc")
            nc.vector.tensor_scalar_add(lc_[:], lens_sb[:], 1e-5)
            rl = p2.tile([BS, 1], F32, tag="rl")
            nc.vector.reciprocal(rl[:], lc_[:])
            cs = p2.tile([BS, 1], F32, tag="cs")
            nc.vector.tensor_tensor(out=cs[:], in0=csum[:], in1=rl[:],
                                    op=mybir.AluOpType.mult)
            gate = p2.tile([BS, 1], F32, tag="gate")
            nc.vector.tensor_scalar(
                out=gate[:], in0=cs[:], scalar1=1e-4, scalar2=None,
                op0=mybir.AluOpType.is_gt,
            )

            # attention output
            attn_t = p2.tile([BS, L], F32, tag="attn_t")
            nc.vector.tensor_scalar_mul(attn_t[:], expv[:], rden[:])
            nc.sync.dma_start(out=attn_o[:, :], in_=attn_t[:])
            nc.sync.dma_start(out=cs_o[:, :], in_=cs[:])

            # ---------- z = sum_l exp[b,l] * x[b,l,:]  (diag matmuls) ----------
            if STAGE >= 3:
                zp = ppz.tile([128, E], F32, tag="zp")
                for c in range(NCHUNK):
                    if c + XG_BUFS < NCHUNK:
                        issue_gather(c + XG_BUFS)
                    xg = xg_tiles[c]
                    for j in range(LC):
                        l = c * LC + j
                        dg = dgp.tile([128, 128], BF16, tag="dg")
                        nc.any.tensor_scalar(
                            out=dg[:], in0=idb[:], scalar1=expv[:, l:l + 1], scalar2=None,
                            op0=mybir.AluOpType.mult,
                        )
                        nc.tensor.matmul(
                            zp[:], lhsT=dg[:], rhs=xg[:, j, :E],
                            start=(l == 0), stop=(l == L - 1),
                        )
                zs = p2.tile([128, E], F32, tag="zs")
                nc.vector.tensor_scalar_mul(zs[:], zp[:], rden[:])
                nc.vector.tensor_scalar_mul(zs[:], zs[:], gate[:])
                nc.sync.dma_start(out=enc_o[:, :], in_=zs[:])
            else:
                zs = p2.tile([128, E], F32, tag="zs")
                nc.vector.memset(zs[:], 0.0)
                if STAGE >= 2:
                    # consume the gathers so their slots/sems resolve
                    for c in range(min(XG_BUFS, NCHUNK)):
                        nc.vector.tensor_copy(zs[:], xg_tiles[c][:, 0, :E])
                nc.sync.dma_start(out=enc_o[:, :], in_=zs[:])

    nc.finalize()
    return nc


_NC = None


def _get_nc():
    global _NC
    if _NC is None:
        _NC = _build()
    return _NC


def _prep_core(ids_core: np.ndarray, w_emb: np.ndarray, shard_f32, shard_t_f32,
               a_emb, a_weight):
    """Host-side layout prep for one core (pure indexing/dtype work)."""
    uniq = np.unique(ids_core)                      # sorted, <= 32768
    assert uniq.shape[0] <= NU
    ut = np.zeros((NU, EP), dtype=NP_BF16)
    ut[:uniq.shape[0], :E] = w_emb[uniq].astype(NP_BF16)

    pos = np.searchsorted(uniq, ids_core)           # [BS, L] positions
    # instance order i = l*128 + b  ->  flat[i] = pos[b, l]
    flat = pos.T.reshape(-1).astype(np.int16)       # [L*BS]
    idx16 = _wrap16(flat)

    flat_m = (ids_core.T.reshape(-1) >> 6).astype(np.int16)   # instance order
    idxm16 = _wrap16(flat_m)
    onehot = (ids_core[..., None] & 63) == np.arange(64)
    onehot = np.ascontiguousarray(onehot.reshape(BS, L * 64).astype(np.float32))
    lens = (ids_core != 0).sum(axis=1).astype(np.float32).reshape(BS, 1)

    return {
        "w_vmaj": shard_f32,
        "w_emaj": shard_t_f32,
        "a_mat": np.ascontiguousarray(a_emb.astype(np.float32)),
        "a_w": np.ascontiguousarray(a_weight.astype(np.float32).reshape(1, A)),
        "ut": ut,
        "idx16": idx16,
        "idxm16": idxm16,
        "oh": onehot,
        "lens": lens,
    }


def kernel(inputs, w_emb, a_emb, a_weight):
    inputs = np.asarray(inputs)
    w_emb = np.ascontiguousarray(np.asarray(w_emb, dtype=np.float32))
    a_emb = np.asarray(a_emb, dtype=np.float32)
    a_weight = np.asarray(a_weight, dtype=np.float32)

    nc = _get_nc()
    in_maps = []
    for c in range(N_CORES):
        ids_core = inputs[c * BS:(c + 1) * BS]
        shard = np.zeros((VS_PAD, E), dtype=np.float32)
        shard[:VS] = w_emb[c * VS:(c + 1) * VS]
        shard_t = np.ascontiguousarray(shard.T)
        in_maps.append(_prep_core(ids_core, w_emb, shard, shard_t, a_emb, a_weight))

    res = run_bass_kernel_spmd(nc, in_maps, core_ids=list(range(N_CORES)))
    enc = np.concatenate([res.results[c]["enc"] for c in range(N_CORES)], axis=0)
    attn = np.concatenate([res.results[c]["attn"] for c in range(N_CORES)], axis=0)
    cs = np.concatenate([res.results[c]["cscore"][:, 0] for c in range(N_CORES)], axis=0)
    return enc, attn, cs


# revision 12
# speedup vs baseline: 1.0569x; 1.0569x over previous
"""AspectMemorySummarizer on 8 TRN2 NeuronCores.

Math (per example b): gather word embeddings x = w_emb[ids[b]], cosine-sim
against 30 aspect vectors, threshold > 0.2, scale by aspect weight, max over
aspects -> per-token score M.  centroid_score = sum(M)/len; attention =
softmax over tokens of (M if M>0 else -1e9); z = attn @ x;
enc = z * (centroid_score > 1e-4).

Key restructuring: M depends only on the vocab id, so it is precomputed per
vocab word (V-sharded across the 8 cores + AllGather), then fetched per token.
The embedding gather uses a per-core deduplicated table (unique ids of the
core's batch shard, <= 32768 rows) so indices fit dma_gather's int16 and rows
can be padded to a 256B multiple in bf16.

Sharding: data-parallel over B for the main phase (128 examples/core);
V-sharded phase 1 (6250 vocab rows/core) + 200KB AllGather.
"""
import sys, os
sys.path.insert(0, "/opt/trn_rl_repo")
import numpy as np
import concourse.bass as bass
import concourse.bacc as bacc
import concourse.mybir as mybir
from concourse.bass import IndirectOffsetOnAxis
from concourse.tile import TileContext
from concourse.bass_utils import run_bass_kernel_spmd

F32 = mybir.dt.float32
BF16 = mybir.dt.bfloat16
I16 = mybir.dt.int16
I32 = mybir.dt.int32
NP_BF16 = mybir.dt.np(BF16)


def _install_ntff_hook_shim():
    """The agent image's antenv lacks axon_hooks; provide it so
    run_bass_kernel_spmd(trace=True) can capture NTFF profiles."""
    import types
    try:
        import antenv
        try:
            from antenv import axon_hooks  # noqa: F401
            return  # already present
        except ImportError:
            pass
        mod = types.ModuleType("antenv.axon_hooks")
        mod._hook = None

        def set_axon_ntff_profile_hook(h):
            mod._hook = h

        def get_axon_ntff_profile_hook():
            return mod._hook

        mod.set_axon_ntff_profile_hook = set_axon_ntff_profile_hook
        mod.get_axon_ntff_profile_hook = get_axon_ntff_profile_hook
        sys.modules["antenv.axon_hooks"] = mod
        antenv.axon_hooks = mod
        from trn_agent_boot.trn_boot import _ntff_profile_via_ctypes
        hook = _ntff_profile_via_ctypes("/opt/axon/libaxon_pjrt.so")
        if hook is not None:
            mod.set_axon_ntff_profile_hook(hook)
    except Exception:
        pass


_install_ntff_hook_shim()


B, L, V, E, A = 1024, 256, 50000, 300, 30
N_CORES = 8
BS = B // N_CORES          # 128 examples per core
VS = V // N_CORES          # 6250 vocab rows per core (phase 1)
VT = 49                    # phase-1 v-tiles of 128 (6272 padded)
VS_PAD = VT * 128
NU = 32768                 # unique-table rows (>= max distinct ids per core)
EP = 384                   # bf16 row padding: 384*2 = 768 B (256B multiple)
NT = BS * L                # 32768 token instances per core
LC = 32                    # l-chunk size for the gather/attention pipeline
NCHUNK = L // LC
XG_BUFS = 4                # gathered-x chunk buffers resident in SBUF

WORD_THRES = 0.2
COS_EPS = 1e-8
MASK_NEG = -80.0           # exp(-80) ~ 1.8e-35: matches softmax(-1e9) to f32
STAGE = int(os.environ.get("K_STAGE", "3"))  # 1: no gathers/z  2: +x-gathers  3: full
SUB = os.environ.get("K_SUB", "")  # "noag": skip AllGather+Mpath; "nom": skip M-gathers only


def _wrap16(flat_idx: np.ndarray) -> np.ndarray:
    """dma_gather index layout: index i lives at [i % 16, i // 16],
    replicated across the 8 gpsimd cores (-> 128 partitions)."""
    n = flat_idx.shape[0]
    arr = flat_idx.reshape(n // 16, 16).T
    return np.tile(arr, (8, 1)).copy()


def _build():
    nc = bacc.Bacc(num_swdge_queues=4)

    # ---- inputs (per core) ----
    w_vmaj = nc.declare_dram_parameter("w_vmaj", [VS_PAD, E], F32, isOutput=False)
    w_emaj = nc.declare_dram_parameter("w_emaj", [E, VS_PAD], F32, isOutput=False)
    a_mat = nc.declare_dram_parameter("a_mat", [A, E], F32, isOutput=False)
    a_w = nc.declare_dram_parameter("a_w", [1, A], F32, isOutput=False)
    ut = nc.declare_dram_parameter("ut", [NU, EP], BF16, isOutput=False)
    idx16 = nc.declare_dram_parameter("idx16", [128, NT // 16], I16, isOutput=False)
    idxm16 = nc.declare_dram_parameter("idxm16", [128, NT // 16], I16, isOutput=False)
    oh = nc.declare_dram_parameter("oh", [BS, L * 64], F32, isOutput=False)
    lens = nc.declare_dram_parameter("lens", [BS, 1], F32, isOutput=False)

    enc_o = nc.declare_dram_parameter("enc", [BS, E], F32, isOutput=True)
    attn_o = nc.declare_dram_parameter("attn", [BS, L], F32, isOutput=True)
    cs_o = nc.declare_dram_parameter("cscore", [BS, 1], F32, isOutput=True)

    m_shard = nc.dram_tensor("m_shard", [VS_PAD], F32)
    m_full = nc.dram_tensor("m_full", [784 * 64], F32)

    ident_f = nc.inline_tensor(np.eye(128, dtype=np.float32), name="ident_f")
    ident_b = nc.inline_tensor(np.eye(128, dtype=NP_BF16), name="ident_b")

    with TileContext(nc) as tc:
        with (
            tc.tile_pool(name="const", bufs=1) as cpool,
            tc.tile_pool(name="p1", bufs=3) as p1,
            tc.tile_pool(name="p1s", bufs=4) as p1s,
            tc.tile_pool(name="xg", bufs=XG_BUFS) as xgp,
            tc.tile_pool(name="p2", bufs=1) as p2,
            tc.tile_pool(name="mb", bufs=2) as mbp,
            tc.tile_pool(name="diag", bufs=4) as dgp,
            tc.tile_pool(name="psum", bufs=2, space="PSUM") as pp,
            tc.tile_pool(name="psz", bufs=1, space="PSUM") as ppz,
        ):
            # ---------- constants ----------
            idf = cpool.tile([128, 128], F32)
            nc.sync.dma_start(out=idf[:], in_=ident_f[:, :])
            idb = cpool.tile([128, 128], BF16)
            nc.sync.dma_start(out=idb[:], in_=ident_b[:, :])
            idx_sb = cpool.tile([128, NT // 16], I16)
            nc.sync.dma_start(out=idx_sb[:], in_=idx16[:, :])
            idxm_sb = cpool.tile([128, NT // 16], I16)
            nc.sync.dma_start(out=idxm_sb[:], in_=idxm16[:, :])
            lens_sb = cpool.tile([BS, 1], F32)
            nc.sync.dma_start(out=lens_sb[:], in_=lens[:, :])

            # ---------- x-gathers (independent of phase 1; issue early) ----------
            # Only XG_BUFS gathers are issued before the collective: a later
            # gather waits on a buffer slot released by z-matmuls, which need
            # M_tok from the indirect gather queued on this same engine --
            # issuing all 8 up front deadlocks the Pool queue.
            xg_tiles = [None] * NCHUNK

            def issue_gather(c):
                xg = xgp.tile([128, LC, EP], BF16, tag="xg")
                nidx = 128 * LC
                nc.gpsimd.dma_gather(
                    xg[:], ut[:, :], idx_sb[:, c * (nidx // 16):(c + 1) * (nidx // 16)],
                    num_idxs=nidx, num_idxs_reg=nidx, elem_size=EP,
                    single_packet=False, queue_num=c % 4,
                )
                xg_tiles[c] = xg

            if STAGE >= 2:
                for c in range(min(XG_BUFS, NCHUNK)):
                    issue_gather(c)

            # ---------- aspect preprocessing ----------
            am = cpool.tile([A, E], F32)
            nc.sync.dma_start(out=am[:], in_=a_mat[:, :])
            n2a = cpool.tile([A, 1], F32)
            sc = cpool.tile([A, E], F32, tag="ascaled")
            nc.vector.tensor_tensor(out=sc[:], in0=am[:], in1=am[:],
                                    op=mybir.AluOpType.mult)
            nc.vector.reduce_sum(n2a[:], sc[:], axis=mybir.AxisListType.X)
            na = cpool.tile([A, 1], F32, tag="na")
            nc.scalar.activation(na[:], n2a[:], mybir.ActivationFunctionType.Sqrt)
            nc.vector.tensor_scalar_max(na[:], na[:], COS_EPS)
            ra = cpool.tile([A, 1], F32, tag="ra")
            nc.vector.reciprocal(ra[:], na[:])
            asc = cpool.tile([A, E], F32, tag="asc")
            nc.vector.tensor_scalar_mul(asc[:], am[:], ra[:])
            # transpose scaled aspects -> asT chunks [e_chunk, A]
            e_chunks = [(0, 128), (128, 128), (256, E - 256)]
            asT = []
            for k, (e0, ew) in enumerate(e_chunks):
                pt = pp.tile([128, A], F32, tag="asT_ps")
                nc.tensor.transpose(pt[:ew, :], asc[:, e0:e0 + ew], idf[:A, :A])
                st = cpool.tile([128, A], F32, tag=f"asT{k}")
                nc.vector.tensor_copy(st[:ew, :], pt[:ew, :])
                asT.append(st)
            # aspect weights replicated across partitions: ones[128,1] @ a_w[1,A]
            aw_sb = cpool.tile([1, A], F32)
            nc.sync.dma_start(out=aw_sb[:], in_=a_w[:, :])
            ones_row = cpool.tile([1, 128], F32, tag="ones_row")
            nc.vector.memset(ones_row[:], 1.0)
            wrep_ps = pp.tile([128, A], F32, tag="wrep_ps")
            nc.tensor.matmul(wrep_ps[:], lhsT=ones_row[:], rhs=aw_sb[:], start=True, stop=True)
            wrep = cpool.tile([128, A], F32, tag="wrep")
            nc.vector.tensor_copy(wrep[:], wrep_ps[:])

            # ---------- phase 1: M per vocab word (V-shard) ----------
            for t in range(VT):
                wv = p1.tile([128, E], F32, tag="wv")
                nc.sync.dma_start(out=wv[:], in_=w_vmaj[t * 128:(t + 1) * 128, :])
                wT = []
                for k, (e0, ew) in enumerate(e_chunks):
                    wt = p1.tile([128, 128], F32, tag=f"wT{k}")
                    nc.sync.dma_start(
                        out=wt[:ew, :], in_=w_emaj[e0:e0 + ew, t * 128:(t + 1) * 128])
                    wT.append(wt)
                n2 = p1s.tile([128, 1], F32, tag="n2")
                scr = p1s.tile([128, E], F32, tag="scr")
                nc.vector.tensor_tensor(out=scr[:], in0=wv[:], in1=wv[:],
                                        op=mybir.AluOpType.mult)
                nc.vector.reduce_sum(n2[:], scr[:], axis=mybir.AxisListType.X)
                xn = p1s.tile([128, 1], F32, tag="xn")
                nc.scalar.activation(xn[:], n2[:], mybir.ActivationFunctionType.Sqrt)
                nc.vector.tensor_scalar_max(xn[:], xn[:], COS_EPS)
                rx = p1s.tile([128, 1], F32, tag="rx")
                nc.vector.reciprocal(rx[:], xn[:])

                dot = pp.tile([128, A], F32, tag="dot")
                for k, (e0, ew) in enumerate(e_chunks):
                    nc.tensor.matmul(
                        dot[:], lhsT=wT[k][:ew, :], rhs=asT[k][:ew, :],
                        start=(k == 0), stop=(k == 2),
                    )
                cosv = p1s.tile([128, A], F32, tag="cosv")
                nc.vector.tensor_scalar_mul(cosv[:], dot[:], rx[:])
                mask = p1s.tile([128, A], F32, tag="mask")
                nc.vector.tensor_scalar(
                    out=mask[:], in0=cosv[:], scalar1=WORD_THRES, scalar2=None,
                    op0=mybir.AluOpType.is_gt,
                )
                tw = p1s.tile([128, A], F32, tag="tw")
                nc.vector.tensor_tensor(out=tw[:], in0=cosv[:], in1=wrep[:],
                                        op=mybir.AluOpType.mult)
                nc.vector.tensor_tensor(out=tw[:], in0=tw[:], in1=mask[:],
                                        op=mybir.AluOpType.mult)
                mt = p1s.tile([128, 1], F32, tag="mt")
                nc.vector.reduce_max(mt[:], tw[:], axis=mybir.AxisListType.X)
                nc.sync.dma_start(
                    out=m_shard[t * 128:(t + 1) * 128].rearrange("(p o) -> p o", p=128),
                    in_=mt[:],
                )

            if SUB != "noag":
                nc.gpsimd.collective_compute(
                    "AllGather", mybir.AluOpType.bypass,
                    replica_groups=[list(range(N_CORES))],
                    ins=[m_shard[0:VS].opt()],
                    outs=[m_full[0:V].opt()],
                )
                zpad = p2.tile([1, 784 * 64 - V], F32, tag="zpad")
                nc.vector.memset(zpad[:], 0.0)
                nc.sync.dma_start(
                    out=m_full[V:784 * 64].rearrange("(p f) -> p f", p=1), in_=zpad[:])

            # ---------- per-token M: gather 64-wide blocks, onehot select ----------
            # (indirect DMA only supports one index per partition row, so a
            # scalar gather is done as block-gather + in-block select)
            m_tok = p2.tile([BS, L], F32, tag="m_tok")
            if SUB in ("noag", "nom"):
                nc.vector.memset(m_tok[:], 0.0)
            MC = 64                      # l's per M-chunk
            m_in = m_full.ap().rearrange("(v c) -> v c", c=64)
            for mc in range(L // MC if SUB not in ("noag", "nom") else 0):
                nidx = 128 * MC
                mblk = mbp.tile([128, MC, 64], F32, tag="mblk")
                nc.gpsimd.dma_gather(
                    mblk[:], m_in,
                    idxm_sb[:, mc * (nidx // 16):(mc + 1) * (nidx // 16)],
                    num_idxs=nidx, num_idxs_reg=nidx, elem_size=64,
                    single_packet=False, queue_num=mc % 4,
                )
                oh_sb = mbp.tile([BS, MC * 64], F32, tag="oh_sb")
                nc.sync.dma_start(
                    out=oh_sb[:], in_=oh[:, mc * MC * 64:(mc + 1) * MC * 64])
                nc.vector.tensor_tensor(
                    out=mblk[:], in0=mblk[:],
                    in1=oh_sb[:].rearrange("p (a b) -> p a b", b=64),
                    op=mybir.AluOpType.mult)
                nc.vector.reduce_max(
                    m_tok[:, mc * MC:(mc + 1) * MC], mblk[:],
                    axis=mybir.AxisListType.X)

            # ---------- softmax pieces ----------
            maskp = p2.tile([BS, L], F32, tag="maskp")
            nc.vector.tensor_scalar(
                out=maskp[:], in0=m_tok[:], scalar1=0.0, scalar2=None,
                op0=mybir.AluOpType.is_gt,
            )
            # score = M + (mask*80 - 80):  masked tokens -> M-80 ~= -80
            adj = p2.tile([BS, L], F32, tag="adj")
            nc.vector.tensor_scalar(
                out=adj[:], in0=maskp[:], scalar1=-MASK_NEG, scalar2=MASK_NEG,
                op0=mybir.AluOpType.mult, op1=mybir.AluOpType.add,
            )
            score = p2.tile([BS, L], F32, tag="score")
            nc.vector.tensor_tensor(out=score[:], in0=m_tok[:], in1=adj[:],
                                    op=mybir.AluOpType.add)
            expv = p2.tile([BS, L], F32, tag="expv")
            den = p2.tile([BS, 1], F32, tag="den")
            nc.scalar.activation(expv[:], score[:], mybir.ActivationFunctionType.Exp,
                                 accum_out=den[:])
            rden = p2.tile([BS, 1], F32, tag="rden")
            nc.vector.reciprocal(rden[:], den[:])
            # centroid score + gate
            csum = p2.tile([BS, 1], F32, tag="csum")
            nc.vector.reduce_sum(csum[:], m_tok[:], axis=mybir.AxisListType.X)
            lc_ = p2.tile([BS, 1], F32, tag="lc")
            nc.vector.tensor_scalar_add(lc_[:], lens_sb[:], 1e-5)
            rl = p2.tile([BS, 1], F32, tag="rl")
            nc.vector.reciprocal(rl[:], lc_[:])
            cs = p2.tile([BS, 1], F32, tag="cs")
            nc.vector.tensor_tensor(out=cs[:], in0=csum[:], in1=rl[:],
                                    op=mybir.AluOpType.mult)
            gate = p2.tile([BS, 1], F32, tag="gate")
            nc.vector.tensor_scalar(
                out=gate[:], in0=cs[:], scalar1=1e-4, scalar2=None,
                op0=mybir.AluOpType.is_gt,
            )

            # attention output
            attn_t = p2.tile([BS, L], F32, tag="attn_t")
            nc.vector.tensor_scalar_mul(attn_t[:], expv[:], rden[:])
            nc.sync.dma_start(out=attn_o[:, :], in_=attn_t[:])
            nc.sync.dma_start(out=cs_o[:, :], in_=cs[:])

            # ---------- z = sum_l exp[b,l] * x[b,l,:]  (diag matmuls) ----------
            if STAGE >= 3:
                expb = p2.tile([BS, L], BF16, tag="expb")
                nc.vector.tensor_copy(expb[:], expv[:])
                zp = ppz.tile([128, E], F32, tag="zp")
                for c in range(NCHUNK):
                    if c + XG_BUFS < NCHUNK:
                        issue_gather(c + XG_BUFS)
                    xg = xg_tiles[c]
                    # xs[b, j, :] = exp[b, c*LC+j] * x[b, j, :]  (in place)
                    eb = expb[:, c * LC:(c + 1) * LC]
                    eb_b = bass.AP(eb.tensor, eb.offset, list(eb.ap) + [[0, EP]])
                    nc.vector.tensor_tensor(
                        out=xg[:], in0=xg[:], in1=eb_b, op=mybir.AluOpType.mult)
                    for j in range(LC):
                        l = c * LC + j
                        nc.tensor.matmul(
                            zp[:], lhsT=idb[:], rhs=xg[:, j, :E],
                            start=(l == 0), stop=(l == L - 1),
                        )
                zs = p2.tile([128, E], F32, tag="zs")
                nc.vector.tensor_scalar_mul(zs[:], zp[:], rden[:])
                nc.vector.tensor_scalar_mul(zs[:], zs[:], gate[:])
                nc.sync.dma_start(out=enc_o[:, :], in_=zs[:])
            else:
                zs = p2.tile([128, E], F32, tag="zs")
                nc.vector.memset(zs[:], 0.0)
                if STAGE >= 2:
                    # consume the gathers so their slots/sems resolve
                    for c in range(min(XG_BUFS, NCHUNK)):
                        nc.vector.tensor_copy(zs[:], xg_tiles[c][:, 0, :E])
                nc.sync.dma_start(out=enc_o[:, :], in_=zs[:])

    nc.finalize()
    return nc


_NC = None


def _get_nc():
    global _NC
    if _NC is None:
        _NC = _build()
    return _NC


def _prep_core(ids_core: np.ndarray, w_emb: np.ndarray, shard_f32, shard_t_f32,
               a_emb, a_weight):
    """Host-side layout prep for one core (pure indexing/dtype work)."""
    uniq = np.unique(ids_core)                      # sorted, <= 32768
    assert uniq.shape[0] <= NU
    ut = np.zeros((NU, EP), dtype=NP_BF16)
    ut[:uniq.shape[0], :E] = w_emb[uniq].astype(NP_BF16)

    pos = np.searchsorted(uniq, ids_core)           # [BS, L] positions
    # instance order i = l*128 + b  ->  flat[i] = pos[b, l]
    flat = pos.T.reshape(-1).astype(np.int16)       # [L*BS]
    idx16 = _wrap16(flat)

    flat_m = (ids_core.T.reshape(-1) >> 6).astype(np.int16)   # instance order
    idxm16 = _wrap16(flat_m)
    onehot = (ids_core[..., None] & 63) == np.arange(64)
    onehot = np.ascontiguousarray(onehot.reshape(BS, L * 64).astype(np.float32))
    lens = (ids_core != 0).sum(axis=1).astype(np.float32).reshape(BS, 1)

    return {
        "w_vmaj": shard_f32,
        "w_emaj": shard_t_f32,
        "a_mat": np.ascontiguousarray(a_emb.astype(np.float32)),
        "a_w": np.ascontiguousarray(a_weight.astype(np.float32).reshape(1, A)),
        "ut": ut,
        "idx16": idx16,
        "idxm16": idxm16,
        "oh": onehot,
        "lens": lens,
    }


def kernel(inputs, w_emb, a_emb, a_weight):
    inputs = np.asarray(inputs)
    w_emb = np.ascontiguousarray(np.asarray(w_emb, dtype=np.float32))
    a_emb = np.asarray(a_emb, dtype=np.float32)
    a_weight = np.asarray(a_weight, dtype=np.float32)

    nc = _get_nc()
    in_maps = []
    for c in range(N_CORES):
        ids_core = inputs[c * BS:(c + 1) * BS]
        shard = np.zeros((VS_PAD, E), dtype=np.float32)
        shard[:VS] = w_emb[c * VS:(c + 1) * VS]
        shard_t = np.ascontiguousarray(shard.T)
        in_maps.append(_prep_core(ids_core, w_emb, shard, shard_t, a_emb, a_weight))

    res = run_bass_kernel_spmd(nc, in_maps, core_ids=list(range(N_CORES)))
    enc = np.concatenate([res.results[c]["enc"] for c in range(N_CORES)], axis=0)
    attn = np.concatenate([res.results[c]["attn"] for c in range(N_CORES)], axis=0)
    cs = np.concatenate([res.results[c]["cscore"][:, 0] for c in range(N_CORES)], axis=0)
    return enc, attn, cs
